# revision 2
# baseline (speedup 1.0000x reference)
"""BinarizedFCLayer forward on 8 trn2 NeuronCores.

    out = X @ sign(W).T      X: [8192, 2048] f32, W: [2048, 2048] f32
                             sign(w) = +1 if w >= 0 else -1

Strategy
--------
Data-parallel over the batch dim of X: core c computes rows
[c*1024, (c+1)*1024) of the output; W is replicated.

Per core (M=1024, K=2048, N=2048 -> 8.6 GFLOP(MAC)):
  * The TensorE contracts over the partition dim, so both operands need K on
    partitions. The host passes X^T shards and W^T (pure layout prep).
  * X^T is cast-DMA'd (SWDGE f32->fp16) into a resident SBUF tile. fp16 keeps
    11 mantissa bits -> output rel err ~2e-4 vs fp32 reference.
  * W^T is cast-DMA'd f32->bf16 (bf16 keeps the f32 exponent range, so
    sign(bf16(w)) == sign(w) for every representable magnitude); DVE then
    binarizes to exact +-1 fp16 via is_ge -> {1,0} -> *2-1.
  * PE: psum[mo] accumulates over 16 k-tiles; 512 matmuls of N=512 at bf16/fp16
    rate (~78.6 TF/s) ~= 109 us/core; input DMA 24 MiB ~= 70 us (overlapped).
  * PSUM -> SBUF copies ride ScalarE (keeps DVE free for binarize);
    stores go out on the HWDGE queue while SWDGE handles inputs.

The walrus build here allows at most ONE sync wait per instruction, so a
post-pass splits any multi-wait instruction into single-wait NoOps on the
same engine placed immediately before it.
"""

import numpy as np

try:
    import concourse.bass as bass
except ImportError:  # harness runs from a bare directory
    import sys
    for p in ("/opt/trn_rl_repo", "/root/.axon_site/_ro/trn_rl_repo"):
        if p not in sys.path:
            sys.path.append(p)
    import concourse.bass as bass

import concourse.mybir as mybir
from concourse.tile import TileContext
from concourse.bass_utils import run_bass_kernel_spmd

P = 128
N_CORES = 8
M_FULL, K, N = 8192, 2048, 2048
M = M_FULL // N_CORES          # 1024 rows of X per core
KT = K // P                    # 16 k-tiles
MT = M // P                    # 8 m-tiles of 128
NCH, NW = 4, 512               # 4 n-chunks of 512 (one PSUM bank each)
MQ, MW = 4, 256                # X arrives in 4 m-quarters of 256 (2 m-tiles)

f32 = mybir.dt.float32
f16 = mybir.dt.float16
bf16 = mybir.dt.bfloat16


def _split_multiwait_instructions(nc: bass.Bass) -> int:
    """walrus codegen rejects >1 sync wait per instruction. Hoist extra waits
    onto fresh single-wait NoOps on the same engine right before the
    offending instruction (same-engine sequential waits are equivalent)."""
    n_split = 0
    for fn in nc.m.functions:
        for blk in fn.blocks:
            out = []
            for inst in blk.instructions:
                si = inst.sync_info
                if si is not None and si.on_wait and len(si.on_wait) > 1:
                    waits = list(si.on_wait)
                    for j, w in enumerate(waits[:-1]):
                        nop = mybir.InstNoOp(
                            name=f"{inst.name}_wsplit{j}", ins=[], outs=[])
                        nop.engine = inst.engine
                        nop.sync_info = mybir.SyncInfo(
                            on_wait=[w], on_update=[])
                        out.append(nop)
                        n_split += 1
                    inst.sync_info = mybir.SyncInfo(
                        on_wait=[waits[-1]],
                        on_update=list(si.on_update or []))
                out.append(inst)
            blk.instructions[:] = out
    return n_split


def _build_nc() -> bass.Bass:
    nc = bass.Bass()
    xt = nc.declare_dram_parameter("xt", [K, M], f32, isOutput=False)
    wt = nc.declare_dram_parameter("wt", [K, N], f32, isOutput=False)
    out = nc.declare_dram_parameter("out", [M, N], f32, isOutput=True)

    xt3 = xt[:].rearrange("(kt p) m -> p kt m", p=P)    # [128, 16, 1024]
    wt3 = wt[:].rearrange("(kt p) n -> p kt n", p=P)    # [128, 16, 2048]
    out3 = out[:].rearrange("(mt p) n -> p mt n", p=P)  # [128, 8, 2048]

    with TileContext(nc) as tc:
        with (
            tc.tile_pool(name="resident", bufs=1) as res_pool,
            tc.tile_pool(name="wq", bufs=3) as wq_pool,
            tc.tile_pool(name="osb", bufs=2) as o_pool,
            tc.tile_pool(name="psum", bufs=8, space="PSUM") as p_pool,
        ):
            xq = res_pool.tile([P, KT, M], f16, tag="xq", name="xq")
            wraw = res_pool.tile([P, KT, N], bf16, tag="wraw", name="wraw")

            # Input casts (SWDGE, fresh destinations -> zero-wait DMAs),
            # ~2 MiB f32 source each, interleaved so PE work unblocks early:
            # W chunk nn arrives as two kt-halves; X as m-quarters.
            def wdma(nn, half):
                ks = slice(half * (KT // 2), (half + 1) * (KT // 2))
                ns = slice(nn * NW, (nn + 1) * NW)
                nc.gpsimd.dma_start(out=wraw[:, ks, ns], in_=wt3[:, ks, ns])

            def xdma(mq):
                ms = slice(mq * MW, (mq + 1) * MW)
                nc.gpsimd.dma_start(out=xq[:, :, ms], in_=xt3[:, :, ms])

            wdma(0, 0); xdma(0); wdma(0, 1); xdma(1)
            wdma(1, 0); xdma(2); wdma(1, 1); xdma(3)
            wdma(2, 0); wdma(2, 1); wdma(3, 0); wdma(3, 1)

            # Binarize each W chunk on DVE as soon as its halves land.
            wqs = []
            for nn in range(NCH):
                wq = wq_pool.tile([P, KT, NW], f16, tag="wq", name=f"wq{nn}")
                nsl = slice(nn * NW, (nn + 1) * NW)
                nc.vector.tensor_scalar(wq[:], wraw[:, :, nsl], 0.0, None,
                                        mybir.AluOpType.is_ge)
                nc.vector.tensor_scalar(wq[:], wq[:], 2.0, -1.0,
                                        mybir.AluOpType.mult,
                                        mybir.AluOpType.add)
                wqs.append(wq)

            # PE: unit (nn, mq) = 2 psum banks, 32 matmuls of N=512.
            for nn in range(NCH):
                osb = o_pool.tile([P, MT, NW], f32, tag="osb", name=f"osb{nn}")
                for mq in range(MQ):
                    psums = [
                        p_pool.tile([P, NW], f32, tag="ps",
                                    name=f"ps{nn}_{mq}_{i}")
                        for i in range(2)
                    ]
                    for kt in range(KT):
                        for mo in range(2):
                            mcol = mq * MW + mo * P
                            nc.tensor.matmul(
                                psums[mo][:],
                                lhsT=xq[:, kt, mcol:mcol + P],
                                rhs=wqs[nn][:, kt, :],
                                start=(kt == 0),
                                stop=(kt == KT - 1),
                            )
                    # PSUM -> SBUF on ScalarE (keeps DVE free; gives the
                    # store a single producer proc)
                    for mo in range(2):
                        nc.scalar.activation(
                            out=osb[:, mq * 2 + mo, :], in_=psums[mo][:],
                            func=mybir.ActivationFunctionType.Copy)
                nc.sync.dma_start(
                    out=out3[:, :, nn * NW:(nn + 1) * NW], in_=osb[:])

    _split_multiwait_instructions(nc)
    return nc


_NC_CACHE = None


def _get_nc() -> bass.Bass:
    global _NC_CACHE
    if _NC_CACHE is None:
        _NC_CACHE = _build_nc()
    return _NC_CACHE


def _run(inputs: dict, trace: bool = False, **kw):
    X = np.asarray(inputs["X"], dtype=np.float32)
    W = np.asarray(inputs["W"], dtype=np.float32)
    assert X.shape == (M_FULL, K) and W.shape == (N, K)

    XT = np.ascontiguousarray(X.T)            # [K, M_FULL]
    WT = np.ascontiguousarray(W.T)            # [K, N]
    in_maps = [
        {"xt": np.ascontiguousarray(XT[:, c * M:(c + 1) * M]), "wt": WT}
        for c in range(N_CORES)
    ]
    res = run_bass_kernel_spmd(
        _get_nc(), in_maps, list(range(N_CORES)), trace=trace, **kw)
    out = np.concatenate([res.results[c]["out"] for c in range(N_CORES)],
                         axis=0)
    return out, res


def kernel(X: np.ndarray, W: np.ndarray) -> np.ndarray:
    out, _ = _run({"X": X, "W": W})
    return out
